# revision 5
# baseline (speedup 1.0000x reference)
"""AUGRU (attention-updated GRU) Trainium2 kernel, data-parallel over 8 NeuronCores.

Problem: T=200, B=4096, D=H=64, ragged lengths sorted descending.
  gi = x_t @ W_ih.T + b_ih ; gh = h @ W_hh.T + b_hh
  r = sigmoid(i_r + h_r); z = sigmoid(i_z + h_z); n = tanh(i_n + r*h_n)
  zw = w_t * z ; h' = (1-zw)*h + zw*n = h + zw*(n-h)
  out[t] = h' where t < len, else 0 ; h frozen beyond len.

Sharding: batch strided across cores (core k gets b = k::8) so the descending
lengths balance. Within a core the 512 rows are split into interleaved halves
(even/odd) stacked on partitions 0:64 / 64:128, feature-major layout:
every on-chip tensor is [128 partitions = 64 feats x 2 halves, cols = batch].
Because lengths are descending, at step t only a prefix of columns is alive
(v_t = ceil(count_t/8) rows; N1=ceil(v_t/2) even cols, N2=floor(v_t/2) odd
cols); dead columns are never computed and the host masks the output.
"""

import os
import numpy as np

import concourse.bass as bass
import concourse.bacc as bacc
import concourse.mybir as mybir
from concourse.tile import TileContext
from concourse.bass_utils import run_bass_kernel_spmd

T, B, D, H = 200, 4096, 64, 64
NCORES = 8
BC = B // NCORES  # 512 batch rows per core
HALF = BC // 2    # 256 columns per half

LAST_RESULT = None  # stashed BassKernelResults for profiling from test harness

f32 = mybir.dt.float32
AF = mybir.ActivationFunctionType
ALU = mybir.AluOpType


def _build_program(N1s, N2s):
    """Build the per-core Bass program. N1s/N2s: per-step live column counts."""
    nc = bacc.Bacc()

    x_d = nc.declare_dram_parameter("x", [T, 128, HALF], f32, isOutput=False)
    w_d = nc.declare_dram_parameter("w", [T, 2, HALF], f32, isOutput=False)
    # Stationary matmul operands, already transposed: lhsT[k, m] = W[m, k]
    wrx_d = nc.declare_dram_parameter("wrx", [D, H], f32, isOutput=False)
    wzx_d = nc.declare_dram_parameter("wzx", [D, H], f32, isOutput=False)
    wnx_d = nc.declare_dram_parameter("wnx", [D, H], f32, isOutput=False)
    wrh_d = nc.declare_dram_parameter("wrh", [H, H], f32, isOutput=False)
    wzh_d = nc.declare_dram_parameter("wzh", [H, H], f32, isOutput=False)
    wnh_d = nc.declare_dram_parameter("wnh", [H, H], f32, isOutput=False)
    # Per-partition bias vectors, duplicated across the two halves: [128, 1]
    br_d = nc.declare_dram_parameter("br", [128, 1], f32, isOutput=False)
    bz_d = nc.declare_dram_parameter("bz", [128, 1], f32, isOutput=False)
    bhn_d = nc.declare_dram_parameter("bhn", [128, 1], f32, isOutput=False)
    bin_d = nc.declare_dram_parameter("bin", [128, 1], f32, isOutput=False)
    # Selector for broadcasting w rows to partitions: sel.T @ w2 ([2,128] lhsT)
    sel_d = nc.declare_dram_parameter("sel", [2, 128], f32, isOutput=False)

    out_d = nc.declare_dram_parameter("out", [T, 128, HALF], f32, isOutput=True)

    with TileContext(nc) as tc:
        with (
            tc.tile_pool(name="const", bufs=1) as cpool,
            tc.tile_pool(name="state", bufs=1) as hpool,
            tc.tile_pool(name="xin", bufs=4) as xpool,
            tc.tile_pool(name="win", bufs=4) as wpool,
            tc.tile_pool(name="work", bufs=3) as spool,
            tc.tile_pool(name="ps", bufs=2, space="PSUM") as ppool,
        ):
            # --- load constants into SBUF once ---
            # each weight is duplicated on partitions 0:64 and 64:128 so the
            # half-2 matmuls (rhs at base partition 64) have a matching lhsT
            wts = {}
            for name, dram in [
                ("wrx", wrx_d), ("wzx", wzx_d), ("wnx", wnx_d),
                ("wrh", wrh_d), ("wzh", wzh_d), ("wnh", wnh_d),
            ]:
                t_ = cpool.tile([128, H], f32, tag=name)
                nc.sync.dma_start(out=t_[0:64, :], in_=dram[:, :])
                nc.sync.dma_start(out=t_[64:128, :], in_=dram[:, :])
                wts[name] = t_
            biases = {}
            for name, dram in [("br", br_d), ("bz", bz_d), ("bhn", bhn_d), ("bin", bin_d)]:
                t_ = cpool.tile([128, 1], f32, tag=name)
                nc.sync.dma_start(out=t_[:, :], in_=dram[:, :])
                biases[name] = t_
            sel_t = cpool.tile([2, 128], f32, tag="sel")
            nc.sync.dma_start(out=sel_t[:, :], in_=sel_d[:, :])

            # --- hidden state ping-pong, [128, HALF] stacked halves ---
            hA = hpool.tile([128, HALF], f32, tag="hA")
            hB = hpool.tile([128, HALF], f32, tag="hB")
            nc.vector.memset(hA[:, :], 0.0)
            nc.vector.memset(hB[:, :], 0.0)
            htiles = [hA, hB]

            for t in range(T):
                N1 = int(N1s[t])
                N2 = int(N2s[t])
                if N1 == 0:
                    break  # lengths are descending: nothing left to do
                h_cur = htiles[t % 2]
                h_nxt = htiles[(t + 1) % 2]

                # --- loads ---
                x_t = xpool.tile([128, HALF], f32, tag="x")
                nc.sync.dma_start(out=x_t[:, 0:N1], in_=x_d[t, :, 0:N1])
                w2_t = wpool.tile([2, HALF], f32, tag="w2")
                nc.sync.dma_start(out=w2_t[:, 0:N1], in_=w_d[t, :, 0:N1])

                # --- matmuls: gate psums, stacked halves ---
                # PSUM tiles pad to whole 2KB banks; pack two [128, HALF]
                # f32 gate tensors per bank as column halves
                prz = ppool.tile([128, 2 * HALF], f32, tag="prz")
                pn = ppool.tile([128, 2 * HALF], f32, tag="pn")
                pwt = ppool.tile([128, HALF], f32, tag="pw")
                pr = prz[:, 0:HALF]
                pz = prz[:, HALF:2 * HALF]
                pni = pn[:, 0:HALF]
                pnh = pn[:, HALF:2 * HALF]
                pw = pwt

                def gate(psum, wx, wh, accum=True):
                    # x-side then h-side (accumulate), half1 then half2
                    nc.tensor.matmul(psum[0:64, 0:N1], lhsT=wts[wx][0:64, :],
                                     rhs=x_t[0:64, 0:N1], start=True, stop=not accum)
                    if N2 > 0:
                        nc.tensor.matmul(psum[64:128, 0:N2], lhsT=wts[wx][64:128, :],
                                         rhs=x_t[64:128, 0:N2], start=True, stop=not accum)
                    if accum:
                        nc.tensor.matmul(psum[0:64, 0:N1], lhsT=wts[wh][0:64, :],
                                         rhs=h_cur[0:64, 0:N1], start=False, stop=True)
                        if N2 > 0:
                            nc.tensor.matmul(psum[64:128, 0:N2], lhsT=wts[wh][64:128, :],
                                             rhs=h_cur[64:128, 0:N2], start=False, stop=True)

                gate(pr, "wrx", "wrh")
                gate(pz, "wzx", "wzh")
                # n gate: keep x-side and h-side separate
                nc.tensor.matmul(pni[0:64, 0:N1], lhsT=wts["wnx"][0:64, :],
                                 rhs=x_t[0:64, 0:N1], start=True, stop=True)
                if N2 > 0:
                    nc.tensor.matmul(pni[64:128, 0:N2], lhsT=wts["wnx"][64:128, :],
                                     rhs=x_t[64:128, 0:N2], start=True, stop=True)
                nc.tensor.matmul(pnh[0:64, 0:N1], lhsT=wts["wnh"][0:64, :],
                                 rhs=h_cur[0:64, 0:N1], start=True, stop=True)
                if N2 > 0:
                    nc.tensor.matmul(pnh[64:128, 0:N2], lhsT=wts["wnh"][64:128, :],
                                     rhs=h_cur[64:128, 0:N2], start=True, stop=True)
                # broadcast attention weights to all partitions: pw = sel.T @ w2
                nc.tensor.matmul(pw[:, 0:N1], lhsT=sel_t[:, :],
                                 rhs=w2_t[:, 0:N1], start=True, stop=True)

                # --- gates ---
                rs = spool.tile([128, HALF], f32, tag="rs")
                zs = spool.tile([128, HALF], f32, tag="zs")
                nc.scalar.activation(rs[:, 0:N1], pr[:, 0:N1], AF.Sigmoid,
                                     bias=biases["br"][:, 0:1], scale=1.0)
                nc.scalar.activation(zs[:, 0:N1], pz[:, 0:N1], AF.Sigmoid,
                                     bias=biases["bz"][:, 0:1], scale=1.0)
                # m1 = (h_n + b_hn) * r
                m1 = spool.tile([128, HALF], f32, tag="m1")
                nc.vector.scalar_tensor_tensor(
                    out=m1[:, 0:N1], in0=pnh[:, 0:N1], scalar=biases["bhn"][:, 0:1],
                    in1=rs[:, 0:N1], op0=ALU.add, op1=ALU.mult)
                a1 = spool.tile([128, HALF], f32, tag="a1")
                nc.vector.tensor_add(a1[:, 0:N1], m1[:, 0:N1], pni[:, 0:N1])
                n_t = spool.tile([128, HALF], f32, tag="nt")
                nc.scalar.activation(n_t[:, 0:N1], a1[:, 0:N1], AF.Tanh,
                                     bias=biases["bin"][:, 0:1], scale=1.0)

                # --- update: h' = h + (w*z)*(n - h) ---
                zw = spool.tile([128, HALF], f32, tag="zw")
                nc.vector.tensor_mul(zw[:, 0:N1], zs[:, 0:N1], pw[:, 0:N1])
                d_t = spool.tile([128, HALF], f32, tag="dt")
                nc.vector.tensor_sub(d_t[:, 0:N1], n_t[:, 0:N1], h_cur[:, 0:N1])
                e_t = spool.tile([128, HALF], f32, tag="et")
                nc.vector.tensor_mul(e_t[:, 0:N1], zw[:, 0:N1], d_t[:, 0:N1])
                nc.vector.tensor_add(h_nxt[:, 0:N1], h_cur[:, 0:N1], e_t[:, 0:N1])

                # --- store ---
                nc.sync.dma_start(out=out_d[t, :, 0:N1], in_=h_nxt[:, 0:N1])

    nc.compile()
    return nc


def kernel(x, weights, lengths, W_ih, W_hh, b_ih, b_hh):
    global LAST_RESULT
    x = np.asarray(x, dtype=np.float32)
    weights = np.asarray(weights, dtype=np.float32)
    lengths = np.asarray(lengths, dtype=np.int32)
    W_ih = np.asarray(W_ih, dtype=np.float32)
    W_hh = np.asarray(W_hh, dtype=np.float32)
    b_ih = np.asarray(b_ih, dtype=np.float32)
    b_hh = np.asarray(b_hh, dtype=np.float32)

    # live row count per step, shared across cores (max over cores = ceil/8)
    counts = (lengths[None, :] > np.arange(T)[:, None]).sum(axis=1)  # [T]
    v = -(-counts // NCORES)          # ceil(counts/8): live cols per core
    N1s = (v + 1) // 2                # even-half cols
    N2s = v // 2                      # odd-half cols

    # x: [T,B,D] -> per-core [T, 128, HALF] (feature-major, interleaved halves)
    xr = x.reshape(T, BC, NCORES, D)            # [t, j, k, d], b = 8j+k
    xr = xr.transpose(2, 0, 3, 1)               # [k, t, d, j]
    xr = xr.reshape(NCORES, T, D, HALF, 2)      # j = 2c + half
    xr = xr.transpose(0, 1, 4, 2, 3)            # [k, t, half, d, c]
    x8 = np.ascontiguousarray(xr.reshape(NCORES, T, 128, HALF))

    wr = weights[:, :, 0].reshape(T, BC, NCORES)   # [t, j, k]
    wr = wr.transpose(2, 0, 1)                     # [k, t, j]
    wr = wr.reshape(NCORES, T, HALF, 2).transpose(0, 1, 3, 2)  # [k, t, half, c]
    w8 = np.ascontiguousarray(wr)

    wts = {
        "wrx": np.ascontiguousarray(W_ih[0:64].T),
        "wzx": np.ascontiguousarray(W_ih[64:128].T),
        "wnx": np.ascontiguousarray(W_ih[128:192].T),
        "wrh": np.ascontiguousarray(W_hh[0:64].T),
        "wzh": np.ascontiguousarray(W_hh[64:128].T),
        "wnh": np.ascontiguousarray(W_hh[128:192].T),
    }
    b_r = (b_ih[0:64] + b_hh[0:64]).astype(np.float32)
    b_z = (b_ih[64:128] + b_hh[64:128]).astype(np.float32)
    b_hn = b_hh[128:192].astype(np.float32)
    b_in = b_ih[128:192].astype(np.float32)
    biases = {
        "br": np.tile(b_r, 2).reshape(128, 1),
        "bz": np.tile(b_z, 2).reshape(128, 1),
        "bhn": np.tile(b_hn, 2).reshape(128, 1),
        "bin": np.tile(b_in, 2).reshape(128, 1),
    }
    sel = np.zeros((2, 128), dtype=np.float32)
    sel[0, 0:64] = 1.0
    sel[1, 64:128] = 1.0

    nc = _build_program(N1s, N2s)

    in_maps = []
    for k in range(NCORES):
        m = {"x": x8[k], "w": w8[k], "sel": sel}
        m.update(wts)
        m.update(biases)
        in_maps.append(m)

    trace = bool(os.environ.get("AUGRU_TRACE"))
    res = run_bass_kernel_spmd(nc, in_maps, list(range(NCORES)), trace=trace)
    LAST_RESULT = res

    outs = np.stack([np.asarray(res.results[k]["out"]) for k in range(NCORES)])
    # [k, t, 128, HALF] -> [T, B, H]
    o = outs.reshape(NCORES, T, 2, H, HALF)      # [k, t, half, d, c]
    o = o.transpose(1, 4, 2, 0, 3)               # [t, c, half, k, d]
    o = o.reshape(T, BC, NCORES, H)              # j = 2c+half
    o = o.reshape(T, B, H)                       # b = 8j+k -> wait, see below
    # Correct assembly: out[t, 8j+k, :] = per-core col j. reshape(T,BC,NCORES,H)
    # with axes [t, j, k, d] flattens to b = j*8 + k which matches b = 8j+k.
    mask = (np.arange(T)[:, None] < lengths[None, :])
    o = np.where(mask[:, :, None], o, np.float32(0.0)).astype(np.float32)
    return o
